# revision 32
# baseline (speedup 1.0000x reference)
"""DFFN Trainium2 kernel for nn_DFFN_81535659147929.

Pipeline: project_in (1x1 conv, 64->340) -> per-8x8-patch rFFT2 * learned
filter -> irFFT2 -> depthwise 3x3 conv -> GELU gate -> project_out (170->64).

Key algebra: the per-patch rFFT2*w->irFFT2 step is, per channel c, a linear
map M_c on the 64 patch pixels, and all M_c are simultaneously diagonalized
by the (channel-independent) orthonormal real 2D-DFT basis C:
M_c = C^T diag(lam_c) C.  So the whole FFT stage becomes two shared-weight
matmuls around a per-(channel,freq) scale.

Sharding: data-parallel, core = (image b = core//2, row half core%2) with an
8-row patch-aligned halo; weights replicated.

Device pipeline per core:
  Phase 1, per 2-patch block (128 pixels on partitions):
    A:   psumA[pix,col] = x_block^T @ w_inT     (project_in, K=64)
    C:   psumB[frq,col] = CC @ sA               (forward transform)
    lam: sB = psumB * lam_tile                  (DVE, psum evac)
    E:   psumZ[col,pix] = sB_chunk^T @ CCrhs    (inverse transform; 4 blocks
         share a PSUM bank, one strided evac per bank -> spatial z bands,
         DMA'd to DRAM zbufs)
  Phase 2 (dwconv+gate), per (16-channel group, 8-row quad):
    partitions = (16 ch, 8 rows); the 3x3 conv becomes 3 column-shifted
    matmuls with a banded row-coupling lhsT (dense, not diagonal), psum-
    accumulated; 8 input rows yield 6 output rows (quads overlap by 2).
    Channel groups alternate x1/x2 so the GELU gate pairs align on
    partitions across the two psum tiles.  g bounced to DRAM.
  Phase 3: project_out (K=176 over two chunks) per 16-row band, DMA out.

Channel column order (NCOL=352): 22 interleaved groups of 16,
group 2m = x1 channels 16m..16m+15, group 2m+1 = x2 (= +170), last pair
padded with zero channels.
"""

import numpy as np
import ml_dtypes

import concourse.bass as bass
import concourse.mybir as mybir
from concourse import bacc, tile
from concourse.bass_utils import run_bass_kernel_spmd

BF16 = mybir.dt.bfloat16
F32 = mybir.dt.float32

DIM = 64
C2 = 340
P = 8
B, H, W = 4, 256, 256
N_CORES = 8
ROWS = H // 2
HALO = P
NCOL = 352          # 22 groups x 16, incl 12 pad channels
NGRP = 22
GCH = 176           # padded gate channels (11 groups x 16)
QSTEP = 6           # output rows per dwconv quad (8 input rows)

_cache = {}


# ----------------------------------------------------------------- host math

def _build_basis():
    rows = []
    seen = set()
    p1, p2 = np.meshgrid(np.arange(P), np.arange(P), indexing="ij")
    for u in range(P):
        for v in range(P):
            if (u, v) in seen:
                continue
            nu, nv = (-u) % P, (-v) % P
            th = 2 * np.pi * (u * p1 + v * p2) / P
            if (nu, nv) == (u, v):
                rows.append((np.cos(th) / 8.0).ravel())
            else:
                seen.add((nu, nv))
                rows.append((np.sqrt(2) / 8.0) * np.cos(th).ravel())
                rows.append((np.sqrt(2) / 8.0) * np.sin(th).ravel())
            seen.add((u, v))
    return np.array(rows, dtype=np.float64)


def _lam_for(fft_w, C):
    basis = C.reshape(64, P, P)
    F = np.fft.rfft2(basis)
    w = fft_w.reshape(C2, 1, P, P // 2 + 1).astype(np.float64)
    r = np.fft.irfft2(F[None] * w, s=(P, P))
    return np.einsum('kpq,ckpq->ck', basis, r)      # [C2, 64]


def _col_to_c2():
    """col (0..351) -> c2 channel or -1 (pad)."""
    cols = np.full(NCOL, -1, np.int64)
    for g in range(NGRP):
        m, half = divmod(g, 2)
        for j in range(16):
            ch = 16 * m + j
            if ch < 170:
                cols[g * 16 + j] = ch + 170 * half
    return cols


def _pix_maps():
    C = _build_basis()
    CCrhs = np.zeros((128, 128))
    for pc2 in range(2):
        for f in range(64):
            for p1 in range(P):
                for p2 in range(P):
                    k = p1 * 16 + pc2 * 8 + p2
                    CCrhs[pc2 * 64 + f, k] = C[f, p1 * 8 + p2]
    return CCrhs.T.copy(), CCrhs, C


def _prep_weights(w_in, w_dw, fft_w, w_out):
    CClhsT, CCrhs, C = _pix_maps()
    lam = _lam_for(fft_w, C)
    cols = _col_to_c2()
    valid = cols >= 0

    w_inT = np.zeros((64, NCOL))
    w_inT[:, valid] = w_in.T[:, cols[valid]]

    lam_t = np.zeros((128, NCOL))
    lam_sel = np.zeros((NCOL, 64))
    lam_sel[valid] = lam[cols[valid]]
    lam_t[:] = np.tile(lam_sel.T, (2, 1))[:128]

    dw = w_dw.reshape(C2, 3, 3)
    # dwconv banded lhsT per (group, dx): [128, 128] blockdiag over 16 ch of
    # T[r_in, o] = w_dw[c, r_in - o, dx], r_in - o in {0,1,2}
    dd = np.zeros((128, NGRP * 3 * 128))
    for g in range(NGRP):
        for dx in range(3):
            blk = np.zeros((128, 128))
            for j in range(16):
                c2 = cols[g * 16 + j]
                if c2 < 0:
                    continue
                for o in range(8):
                    for dy in range(3):
                        ri = o + dy
                        if ri < 8:
                            blk[j * 8 + ri, j * 8 + o] = dw[c2, dy, dx]
            dd[:, (g * 3 + dx) * 128:(g * 3 + dx + 1) * 128] = blk

    # project_out lhsT: cols 0:64 = gate-ch 0..127, cols 64:128 = 128..175
    wo = np.zeros((128, 128))
    wo[0:128, 0:64] = w_out.T[0:128]
    wo[0:42, 64:128] = w_out.T[128:170]
    bf = ml_dtypes.bfloat16
    return {
        "w_inT": w_inT.astype(bf),
        "cclhsT": CClhsT.astype(bf),
        "ccrhs": CCrhs.astype(bf),
        "lam_t": lam_t.astype(bf),
        "dd": dd.astype(bf),
        "wo": wo.astype(bf),
    }


# ---------------------------------------------------------------- bass build

def build_nc(rows=ROWS, dbg=False):
    rh = rows + 2 * HALO
    npr = rh // P
    nband = rows // 16
    nquad = (rows + QSTEP - 1) // QSTEP          # 22 for rows=128
    assert rows % 16 == 0

    nc = bacc.Bacc("TRN2", target_bir_lowering=False, debug=False,
                   num_devices=N_CORES)
    x_d = nc.dram_tensor("x", [DIM, rh * W], BF16, kind="ExternalInput")
    winT_d = nc.dram_tensor("w_inT", [64, NCOL], BF16, kind="ExternalInput")
    cclhsT_d = nc.dram_tensor("cclhsT", [128, 128], BF16, kind="ExternalInput")
    ccrhs_d = nc.dram_tensor("ccrhs", [128, 128], BF16, kind="ExternalInput")
    lam_d = nc.dram_tensor("lam_t", [128, NCOL], BF16, kind="ExternalInput")
    dd_d = nc.dram_tensor("dd", [128, NGRP * 3 * 128], BF16,
                          kind="ExternalInput")
    wo_d = nc.dram_tensor("wo", [128, 128], BF16, kind="ExternalInput")
    out_d = nc.dram_tensor("out", [DIM, rows * W], F32, kind="ExternalOutput")

    # z bounce buffers, one per 128-col chunk of the NCOL channel columns
    zkind = "ExternalOutput" if dbg else "Internal"
    zbufs = [nc.dram_tensor(f"zbuf{i}", [128, rh * W], BF16, kind=zkind)
             for i in range(3)]          # chunks: 128, 128, 96 rows used
    gbuf_d = nc.dram_tensor("gbuf", [128, 11 * nquad * W], BF16, kind=zkind)

    G = mybir.ActivationFunctionType.Gelu
    chunks = [(0, 128), (128, 128), (256, 96)]

    with tile.TileContext(nc) as tc:
        with tc.tile_pool(name="consts", bufs=1) as cpool:
            w_inT = cpool.tile([64, NCOL], BF16)
            nc.sync.dma_start(out=w_inT[:], in_=winT_d[:])
            cclhsT = cpool.tile([128, 128], BF16)
            nc.sync.dma_start(out=cclhsT[:], in_=cclhsT_d[:])
            ccrhs = cpool.tile([128, 128], BF16)
            nc.sync.dma_start(out=ccrhs[:], in_=ccrhs_d[:])
            lam_t = cpool.tile([128, NCOL], BF16)
            nc.sync.dma_start(out=lam_t[:], in_=lam_d[:])
            dd = cpool.tile([128, NGRP * 3 * 128], BF16)
            nc.sync.dma_start(out=dd[:], in_=dd_d[:])
            wo = cpool.tile([128, 128], BF16)
            nc.sync.dma_start(out=wo[:], in_=wo_d[:])

            # ---------------- all pools coexist: PSUM adds up to 8 banks
            dxs = [(1, 0, 0, 256), (0, 0, 1, 255), (2, 1, 0, 255)]
            gq = gbuf_d[:].rearrange("(j o) (m q w) -> j m q o w",
                                     j=16, o=8, m=11, q=nquad, w=W)
            npair = (nquad + 1) // 2
            with (
                tc.tile_pool(name="p1x", bufs=1) as xpool,
                tc.tile_pool(name="p1s", bufs=4) as spool,
                tc.tile_pool(name="p1sb", bufs=8) as sbpool,
                tc.tile_pool(name="p1z", bufs=2) as zpool,
                tc.tile_pool(name="p1ps", bufs=2, space="PSUM") as pspool,
                tc.tile_pool(name="p1pz", bufs=2, space="PSUM") as pzpool,
                tc.tile_pool(name="p2z", bufs=4) as zqpool,
                tc.tile_pool(name="p2g", bufs=4) as gpool,
                tc.tile_pool(name="p2ps", bufs=1, space="PSUM") as qpool,
                tc.tile_pool(name="p3g", bufs=2) as grpool,
                tc.tile_pool(name="p3o", bufs=1) as opool,
                tc.tile_pool(name="p3ps", bufs=2, space="PSUM") as popool,
            ):
                # x arrives host-patchified: [64, (pr, pcp, p1, pc2, p2)]
                x_sb = xpool.tile([64, rh * W], BF16)
                nc.sync.dma_start(out=x_sb[:], in_=x_d[:])

                def emit_band(band):
                    prs = [2 * band, 2 * band + 1]
                    zbs = [zpool.tile([128, 16 * W], BF16, tag=f"zb{i}",
                                      name=f"zb{i}")
                           for i in range(3)]
                    for bi, pr in enumerate(prs):
                        for pq in range(4):          # 4 blocks of 4 pcp
                            sBs = []
                            for pj in range(4):
                                pcp = pq * 4 + pj
                                psA = pspool.tile([128, NCOL], F32, tag="pAB")
                                xblk = x_sb[:, (pr * 16 + pcp) * 128:
                                            (pr * 16 + pcp + 1) * 128]
                                nc.tensor.matmul(psA[:], xblk, w_inT[:],
                                                 start=True, stop=True)
                                sA = spool.tile([128, NCOL], BF16, tag="sA")
                                if pj % 2 == 0:
                                    nc.scalar.copy(sA[:], psA[:])
                                else:
                                    nc.vector.tensor_copy(sA[:], psA[:])
                                psB = pspool.tile([128, NCOL], F32, tag="pAB",
                                                  name="psB")
                                nc.tensor.matmul(psB[:], cclhsT[:], sA[:],
                                                 start=True, stop=True)
                                sB = sbpool.tile([128, NCOL], BF16, tag="sB")
                                nc.vector.tensor_mul(sB[:], psB[:], lam_t[:])
                                sBs.append(sB)
                            for ci, (c0, m) in enumerate(chunks):
                                psZ = pzpool.tile([128, 512], F32,
                                                  tag="psZ")
                                for pj in range(4):
                                    nc.tensor.matmul(
                                        psZ[0:m, pj * 128:(pj + 1) * 128],
                                        sBs[pj][:, c0:c0 + m], ccrhs[:],
                                        start=(pj == 0), stop=(pj == 3))
                                # evac: psum cols (pj, p1, pc2p2) read as
                                # (p1, pj, pc2p2) -> zband rows/cols
                                src = psZ[:].rearrange(
                                    "c (pj p1 q) -> c p1 pj q",
                                    pj=4, p1=8, q=16)[0:m]
                                dstr = zbs[ci][:].rearrange(
                                    "c (r pcp q) -> c r pcp q",
                                    r=16, pcp=16, q=16)
                                dst = dstr[0:m, bi * 8:bi * 8 + 8,
                                           pq * 4:pq * 4 + 4, :]
                                if (pq + ci) % 2 == 0:
                                    nc.scalar.copy(dst, src)
                                else:
                                    nc.vector.tensor_copy(dst, src)
                    r0 = 2 * band * 8 * W
                    for i in range(3):
                        nc.sync.dma_start(out=zbufs[i][:, r0:r0 + 16 * W],
                                          in_=zbs[i][:])

                # ------ phases 2+3 interleaved over quad-pair groups
                def emit_qg(qg):
                    q0 = 2 * qg
                    quads = [q for q in (q0, q0 + 1) if q < nquad]
                    nq = len(quads)
                    for m in range(11):              # gate pair m
                        pss = []
                        for half in range(2):
                            g = 2 * m + half
                            ci, roff = g // 8, (g % 8) * 16
                            zt2 = zqpool.tile([128, 2 * W], BF16,
                                              tag=f"zq{half}",
                                              name=f"zq{half}")
                            zsrc = zbufs[ci][:].rearrange(
                                "c (r w) -> c r w", r=rh, w=W)
                            eng = nc.gpsimd
                            for qi, q in enumerate(quads):
                                eng.dma_start(
                                    out=zt2[:, qi * W:(qi + 1) * W],
                                    in_=zsrc[roff:roff + 16,
                                             QSTEP * q + 7:QSTEP * q + 15, :])
                            ps = qpool.tile([128, 2 * W], F32,
                                            tag=f"ps{half}", name=f"ps{half}")
                            zr = zt2[:].rearrange("c (q w) -> c q w", q=2, w=W)
                            pr = ps[:].rearrange("c (q w) -> c q w", q=2, w=W)
                            for k, (dx, wi0, wo0, wn) in enumerate(dxs):
                                lhs = dd[:, (g * 3 + dx) * 128:
                                         (g * 3 + dx + 1) * 128]
                                nc.tensor.matmul(
                                    pr[:, 0:nq, wo0:wo0 + wn], lhs,
                                    zr[:, 0:nq, wi0:wi0 + wn],
                                    start=(k == 0), stop=(k == 2))
                            pss.append(ps)
                        ge = gpool.tile([128, 2 * W], BF16, tag="ge")
                        nc.scalar.activation(ge[:, 0:nq * W],
                                             pss[0][:, 0:nq * W], G)
                        gt2 = gpool.tile([128, 2 * W], BF16, tag="gt2")
                        nc.vector.tensor_mul(gt2[:, 0:nq * W],
                                             ge[:, 0:nq * W],
                                             pss[1][:, 0:nq * W])
                        nc.sync.dma_start(
                            out=gbuf_d[:, (m * nquad + q0) * W:
                                       (m * nquad + q0 + nq) * W],
                            in_=gt2[:, 0:nq * W])

                    # ---- phase 3 for this quad pair
                    ncols = nq * QSTEP * W
                    nrows = min(nq * QSTEP, rows - QSTEP * q0)
                    gA = grpool.tile([128, 2 * QSTEP * W], BF16, tag="gA")
                    gB = grpool.tile([48, 2 * QSTEP * W], BF16, tag="gB")
                    for mm in range(8):
                        for qi in range(nq):
                            nc.sync.dma_start(
                                out=gA[16 * mm:16 * mm + 16,
                                       qi * QSTEP * W:(qi + 1) * QSTEP * W],
                                in_=gq[:, mm, q0 + qi, 0:QSTEP, :])
                    for mm in range(3):
                        for qi in range(nq):
                            nc.sync.dma_start(
                                out=gB[16 * mm:16 * mm + 16,
                                       qi * QSTEP * W:(qi + 1) * QSTEP * W],
                                in_=gq[:, 8 + mm, q0 + qi, 0:QSTEP, :])
                    obnd = opool.tile([64, 2 * QSTEP * W], F32, tag="oband")
                    nct = (ncols + 511) // 512
                    for ct in range(nct):
                        c0, c1 = ct * 512, min((ct + 1) * 512, ncols)
                        po = popool.tile([64, 512], F32, tag="po")
                        nc.tensor.matmul(po[:, 0:c1 - c0], wo[0:128, 0:64],
                                         gA[:, c0:c1],
                                         start=True, stop=False)
                        nc.tensor.matmul(po[:, 0:c1 - c0], wo[0:48, 64:128],
                                         gB[:, c0:c1],
                                         start=False, stop=True)
                        nc.vector.tensor_copy(obnd[:, c0:c1],
                                              po[:, 0:c1 - c0])
                    nc.sync.dma_start(
                        out=out_d[:, QSTEP * q0 * W:
                                  (QSTEP * q0 + nrows) * W],
                        in_=obnd[:, 0:nrows * W])

                # ------ interleaved emission: band b, then ready quad-pairs
                nbands = npr // 2
                ready = {}
                for k in range(npair):
                    quads_k = [q for q in (2 * k, 2 * k + 1) if q < nquad]
                    maxrow = QSTEP * quads_k[-1] + 14
                    b = min(nbands - 1, maxrow // 16)
                    ready.setdefault(b, []).append(k)
                for band in range(nbands):
                    emit_band(band)
                    for k in ready.get(band, []):
                        emit_qg(k)

    nc.compile()
    return nc


# ----------------------------------------------------------------- interface

def _get_program(rows=ROWS):
    key = ("nc", rows)
    if key not in _cache:
        _cache[key] = build_nc(rows)
    return _cache[key]


def _patchify(xs):
    rh = xs.shape[1]
    xp = xs.reshape(DIM, rh // 8, 8, 16, 2, 8).transpose(0, 1, 3, 2, 4, 5)
    return np.ascontiguousarray(xp).reshape(DIM, rh * W).astype(
        ml_dtypes.bfloat16)


def _shard_x(x, rows=ROWS):
    rh = rows + 2 * HALO
    shards = []
    for c in range(N_CORES):
        b, hh = divmod(c, 2)
        r0 = hh * rows
        xs = np.zeros((DIM, rh, W), np.float32)
        lo, hi = r0 - HALO, r0 + rows + HALO
        slo, shi = max(lo, 0), min(hi, x.shape[2])
        xs[:, slo - lo:shi - lo] = x[b, :, slo:shi]
        shards.append(_patchify(xs))
    return shards


def _run(x, w_in, w_dw, fft_w, w_out, trace=False):
    nc = _get_program()
    wts = _prep_weights(np.asarray(w_in, np.float32),
                        np.asarray(w_dw, np.float32).reshape(C2, 3, 3),
                        np.asarray(fft_w, np.float32),
                        np.asarray(w_out, np.float32))
    shards = _shard_x(np.asarray(x, np.float32))
    in_maps = [{"x": s, **wts} for s in shards]
    res = run_bass_kernel_spmd(nc, in_maps, core_ids=list(range(N_CORES)),
                               trace=trace)
    out = np.zeros((B, DIM, H, W), np.float32)
    for c in range(N_CORES):
        b, hh = divmod(c, 2)
        out[b, :, hh * ROWS:(hh + 1) * ROWS] = (
            res.results[c]["out"].reshape(DIM, ROWS, W))
    return out, res.exec_time_ns


def kernel(x, w_in, w_dw, fft_w, w_out):
    out, _ = _run(x, w_in, w_dw, fft_w, w_out, trace=False)
    return out


# revision 33
# speedup vs baseline: 1.1209x; 1.1209x over previous
"""DFFN Trainium2 kernel for nn_DFFN_81535659147929.

Pipeline: project_in (1x1 conv, 64->340) -> per-8x8-patch rFFT2 * learned
filter -> irFFT2 -> depthwise 3x3 conv -> GELU gate -> project_out (170->64).

Key algebra: the per-patch rFFT2*w->irFFT2 step is, per channel c, a linear
map M_c on the 64 patch pixels, and all M_c are simultaneously diagonalized
by the (channel-independent) orthonormal real 2D-DFT basis C:
M_c = C^T diag(lam_c) C.  So the whole FFT stage becomes two shared-weight
matmuls around a per-(channel,freq) scale.

Sharding: data-parallel, core = (image b = core//2, row half core%2) with an
8-row patch-aligned halo; weights replicated.

Device pipeline per core:
  Phase 1, per 2-patch block (128 pixels on partitions):
    A:   psumA[pix,col] = x_block^T @ w_inT     (project_in, K=64)
    C:   psumB[frq,col] = CC @ sA               (forward transform)
    lam: sB = psumB * lam_tile                  (DVE, psum evac)
    E:   psumZ[col,pix] = sB_chunk^T @ CCrhs    (inverse transform; 4 blocks
         share a PSUM bank, one strided evac per bank -> spatial z bands,
         DMA'd to DRAM zbufs)
  Phase 2 (dwconv+gate), per (16-channel group, 8-row quad):
    partitions = (16 ch, 8 rows); the 3x3 conv becomes 3 column-shifted
    matmuls with a banded row-coupling lhsT (dense, not diagonal), psum-
    accumulated; 8 input rows yield 6 output rows (quads overlap by 2).
    Channel groups alternate x1/x2 so the GELU gate pairs align on
    partitions across the two psum tiles.  g bounced to DRAM.
  Phase 3: project_out (K=176 over two chunks) per 16-row band, DMA out.

Channel column order (NCOL=352): 22 interleaved groups of 16,
group 2m = x1 channels 16m..16m+15, group 2m+1 = x2 (= +170), last pair
padded with zero channels.
"""

import numpy as np
import ml_dtypes

import concourse.bass as bass
import concourse.mybir as mybir
from concourse import bacc, tile
from concourse.bass_utils import run_bass_kernel_spmd

BF16 = mybir.dt.bfloat16
F32 = mybir.dt.float32

DIM = 64
C2 = 340
P = 8
B, H, W = 4, 256, 256
N_CORES = 8
ROWS = H // 2
HALO = P
NCOL = 352          # 22 groups x 16, incl 12 pad channels
NGRP = 22
GCH = 176           # padded gate channels (11 groups x 16)
QSTEP = 6           # output rows per dwconv quad (8 input rows)

_cache = {}


# ----------------------------------------------------------------- host math

def _build_basis():
    rows = []
    seen = set()
    p1, p2 = np.meshgrid(np.arange(P), np.arange(P), indexing="ij")
    for u in range(P):
        for v in range(P):
            if (u, v) in seen:
                continue
            nu, nv = (-u) % P, (-v) % P
            th = 2 * np.pi * (u * p1 + v * p2) / P
            if (nu, nv) == (u, v):
                rows.append((np.cos(th) / 8.0).ravel())
            else:
                seen.add((nu, nv))
                rows.append((np.sqrt(2) / 8.0) * np.cos(th).ravel())
                rows.append((np.sqrt(2) / 8.0) * np.sin(th).ravel())
            seen.add((u, v))
    return np.array(rows, dtype=np.float64)


def _lam_for(fft_w, C):
    basis = C.reshape(64, P, P)
    F = np.fft.rfft2(basis)
    w = fft_w.reshape(C2, 1, P, P // 2 + 1).astype(np.float64)
    r = np.fft.irfft2(F[None] * w, s=(P, P))
    return np.einsum('kpq,ckpq->ck', basis, r)      # [C2, 64]


def _col_to_c2():
    """col (0..351) -> c2 channel or -1 (pad)."""
    cols = np.full(NCOL, -1, np.int64)
    for g in range(NGRP):
        m, half = divmod(g, 2)
        for j in range(16):
            ch = 16 * m + j
            if ch < 170:
                cols[g * 16 + j] = ch + 170 * half
    return cols


def _pix_maps():
    C = _build_basis()
    CCrhs = np.zeros((128, 128))
    for pc2 in range(2):
        for f in range(64):
            for p1 in range(P):
                for p2 in range(P):
                    k = p1 * 16 + pc2 * 8 + p2
                    CCrhs[pc2 * 64 + f, k] = C[f, p1 * 8 + p2]
    return CCrhs.T.copy(), CCrhs, C


def _prep_weights(w_in, w_dw, fft_w, w_out):
    CClhsT, CCrhs, C = _pix_maps()
    lam = _lam_for(fft_w, C)
    cols = _col_to_c2()
    valid = cols >= 0

    w_inT = np.zeros((64, NCOL))
    w_inT[:, valid] = w_in.T[:, cols[valid]]

    lam_t = np.zeros((128, NCOL))
    lam_sel = np.zeros((NCOL, 64))
    lam_sel[valid] = lam[cols[valid]]
    lam_t[:] = np.tile(lam_sel.T, (2, 1))[:128]

    dw = w_dw.reshape(C2, 3, 3)
    # dwconv banded lhsT per (group, dx): [128, 128] blockdiag over 16 ch of
    # T[r_in, o] = w_dw[c, r_in - o, dx], r_in - o in {0,1,2}
    dd = np.zeros((128, NGRP * 3 * 128))
    for g in range(NGRP):
        for dx in range(3):
            blk = np.zeros((128, 128))
            for j in range(16):
                c2 = cols[g * 16 + j]
                if c2 < 0:
                    continue
                for o in range(8):
                    for dy in range(3):
                        ri = o + dy
                        if ri < 8:
                            blk[j * 8 + ri, j * 8 + o] = dw[c2, dy, dx]
            dd[:, (g * 3 + dx) * 128:(g * 3 + dx + 1) * 128] = blk

    # project_out lhsT: cols 0:64 = gate-ch 0..127, cols 64:128 = 128..175
    wo = np.zeros((128, 128))
    wo[0:128, 0:64] = w_out.T[0:128]
    wo[0:42, 64:128] = w_out.T[128:170]
    bf = ml_dtypes.bfloat16
    return {
        "w_inT": w_inT.astype(bf),
        "cclhsT": CClhsT.astype(bf),
        "ccrhs": CCrhs.astype(bf),
        "lam_t": lam_t.astype(bf),
        "dd": dd.astype(bf),
        "wo": wo.astype(bf),
    }


# ---------------------------------------------------------------- bass build

def build_nc(rows=ROWS, dbg=False):
    rh = rows + 2 * HALO
    npr = rh // P
    nband = rows // 16
    nquad = (rows + QSTEP - 1) // QSTEP          # 22 for rows=128
    assert rows % 16 == 0

    nc = bacc.Bacc("TRN2", target_bir_lowering=False, debug=False,
                   num_devices=N_CORES)
    x_d = nc.dram_tensor("x", [DIM, rh * W], BF16, kind="ExternalInput")
    winT_d = nc.dram_tensor("w_inT", [64, NCOL], BF16, kind="ExternalInput")
    cclhsT_d = nc.dram_tensor("cclhsT", [128, 128], BF16, kind="ExternalInput")
    ccrhs_d = nc.dram_tensor("ccrhs", [128, 128], BF16, kind="ExternalInput")
    lam_d = nc.dram_tensor("lam_t", [128, NCOL], BF16, kind="ExternalInput")
    dd_d = nc.dram_tensor("dd", [128, NGRP * 3 * 128], BF16,
                          kind="ExternalInput")
    wo_d = nc.dram_tensor("wo", [128, 128], BF16, kind="ExternalInput")
    out_d = nc.dram_tensor("out", [DIM, rows * W], F32, kind="ExternalOutput")

    # z bounce buffers, one per 128-col chunk of the NCOL channel columns
    zkind = "ExternalOutput" if dbg else "Internal"
    zbufs = [nc.dram_tensor(f"zbuf{i}", [128, rh * W], BF16, kind=zkind)
             for i in range(3)]          # chunks: 128, 128, 96 rows used
    gbuf_d = nc.dram_tensor("gbuf", [128, 11 * nquad * W], BF16, kind=zkind)

    G = mybir.ActivationFunctionType.Gelu
    chunks = [(0, 128), (128, 128), (256, 96)]

    with tile.TileContext(nc) as tc:
        with tc.tile_pool(name="consts", bufs=1) as cpool:
            w_inT = cpool.tile([64, NCOL], BF16)
            nc.sync.dma_start(out=w_inT[:], in_=winT_d[:])
            cclhsT = cpool.tile([128, 128], BF16)
            nc.sync.dma_start(out=cclhsT[:], in_=cclhsT_d[:])
            ccrhs = cpool.tile([128, 128], BF16)
            nc.sync.dma_start(out=ccrhs[:], in_=ccrhs_d[:])
            lam_t = cpool.tile([128, NCOL], BF16)
            nc.sync.dma_start(out=lam_t[:], in_=lam_d[:])
            dd = cpool.tile([128, NGRP * 3 * 128], BF16)
            nc.sync.dma_start(out=dd[:], in_=dd_d[:])
            wo = cpool.tile([128, 128], BF16)
            nc.sync.dma_start(out=wo[:], in_=wo_d[:])

            # ---------------- all pools coexist: PSUM adds up to 8 banks
            dxs = [(1, 0, 0, 256), (0, 0, 1, 255), (2, 1, 0, 255)]
            gq = gbuf_d[:].rearrange("(j o) (m q w) -> j m q o w",
                                     j=16, o=8, m=11, q=nquad, w=W)
            npair = (nquad + 1) // 2
            with (
                tc.tile_pool(name="p1x", bufs=1) as xpool,
                tc.tile_pool(name="p1s", bufs=4) as spool,
                tc.tile_pool(name="p1sb", bufs=8) as sbpool,
                tc.tile_pool(name="p1z", bufs=2) as zpool,
                tc.tile_pool(name="p1ps", bufs=2, space="PSUM") as pspool,
                tc.tile_pool(name="p1pz", bufs=2, space="PSUM") as pzpool,
                tc.tile_pool(name="p2z", bufs=4) as zqpool,
                tc.tile_pool(name="p2g", bufs=4) as gpool,
                tc.tile_pool(name="p2ps", bufs=1, space="PSUM") as qpool,
                tc.tile_pool(name="p3g", bufs=2) as grpool,
                tc.tile_pool(name="p3o", bufs=1) as opool,
                tc.tile_pool(name="p3ps", bufs=2, space="PSUM") as popool,
            ):
                # x arrives host-patchified: [64, (pr, pcp, p1, pc2, p2)]
                x_sb = xpool.tile([64, rh * W], BF16)
                nc.sync.dma_start(out=x_sb[:], in_=x_d[:])

                def emit_band(band):
                    prs = [2 * band, 2 * band + 1]
                    zbs = [zpool.tile([128, 16 * W], BF16, tag=f"zb{i}",
                                      name=f"zb{i}")
                           for i in range(3)]
                    for bi, pr in enumerate(prs):
                        for pq in range(4):          # 4 blocks of 4 pcp
                            sBs = []
                            for pj in range(4):
                                pcp = pq * 4 + pj
                                psA = pspool.tile([128, NCOL], F32, tag="pAB")
                                xblk = x_sb[:, (pr * 16 + pcp) * 128:
                                            (pr * 16 + pcp + 1) * 128]
                                nc.tensor.matmul(psA[:], xblk, w_inT[:],
                                                 start=True, stop=True)
                                sA = spool.tile([128, NCOL], BF16, tag="sA")
                                if pj % 2 == 0:
                                    nc.scalar.copy(sA[:], psA[:])
                                else:
                                    nc.vector.tensor_copy(sA[:], psA[:])
                                psB = pspool.tile([128, NCOL], F32, tag="pAB",
                                                  name="psB")
                                nc.tensor.matmul(psB[:], cclhsT[:], sA[:],
                                                 start=True, stop=True)
                                sB = sbpool.tile([128, NCOL], BF16, tag="sB")
                                nc.vector.tensor_mul(sB[:], psB[:], lam_t[:])
                                sBs.append(sB)
                            for ci, (c0, m) in enumerate(chunks):
                                psZ = pzpool.tile([128, 512], F32,
                                                  tag="psZ")
                                for pj in range(4):
                                    nc.tensor.matmul(
                                        psZ[0:m, pj * 128:(pj + 1) * 128],
                                        sBs[pj][:, c0:c0 + m], ccrhs[:],
                                        start=(pj == 0), stop=(pj == 3))
                                # evac: psum cols (pj, p1, pc2p2) read as
                                # (p1, pj, pc2p2) -> zband rows/cols
                                src = psZ[:].rearrange(
                                    "c (pj p1 q) -> c p1 pj q",
                                    pj=4, p1=8, q=16)[0:m]
                                dstr = zbs[ci][:].rearrange(
                                    "c (r pcp q) -> c r pcp q",
                                    r=16, pcp=16, q=16)
                                dst = dstr[0:m, bi * 8:bi * 8 + 8,
                                           pq * 4:pq * 4 + 4, :]
                                if (pq + ci) % 2 == 0:
                                    nc.scalar.copy(dst, src)
                                else:
                                    nc.vector.tensor_copy(dst, src)
                    r0 = 2 * band * 8 * W
                    for i in range(3):
                        nc.sync.dma_start(out=zbufs[i][:, r0:r0 + 16 * W],
                                          in_=zbs[i][:])

                # ------ phases 2+3 interleaved over quad-pair groups
                def emit_qg(qg):
                    q0 = 2 * qg
                    quads = [q for q in (q0, q0 + 1) if q < nquad]
                    nq = len(quads)
                    for m in range(11):              # gate pair m
                        pss = []
                        for half in range(2):
                            g = 2 * m + half
                            ci, roff = g // 8, (g % 8) * 16
                            zt2 = zqpool.tile([128, 2 * W], BF16,
                                              tag=f"zq{half}",
                                              name=f"zq{half}")
                            zsrc = zbufs[ci][:].rearrange(
                                "c (r w) -> c r w", r=rh, w=W)
                            eng = nc.gpsimd
                            for qi, q in enumerate(quads):
                                eng.dma_start(
                                    out=zt2[:, qi * W:(qi + 1) * W],
                                    in_=zsrc[roff:roff + 16,
                                             QSTEP * q + 7:QSTEP * q + 15, :])
                            ps = qpool.tile([128, 2 * W], F32,
                                            tag=f"ps{half}", name=f"ps{half}")
                            zr = zt2[:].rearrange("c (q w) -> c q w", q=2, w=W)
                            pr = ps[:].rearrange("c (q w) -> c q w", q=2, w=W)
                            for k, (dx, wi0, wo0, wn) in enumerate(dxs):
                                lhs = dd[:, (g * 3 + dx) * 128:
                                         (g * 3 + dx + 1) * 128]
                                nc.tensor.matmul(
                                    pr[:, 0:nq, wo0:wo0 + wn], lhs,
                                    zr[:, 0:nq, wi0:wi0 + wn],
                                    start=(k == 0), stop=(k == 2))
                            pss.append(ps)
                        ge = gpool.tile([128, 2 * W], BF16, tag="ge")
                        nc.scalar.activation(ge[:, 0:nq * W],
                                             pss[0][:, 0:nq * W], G)
                        gt2 = gpool.tile([128, 2 * W], BF16, tag="gt2")
                        nc.vector.tensor_mul(gt2[:, 0:nq * W],
                                             ge[:, 0:nq * W],
                                             pss[1][:, 0:nq * W])
                        nc.sync.dma_start(
                            out=gbuf_d[:, (m * nquad + q0) * W:
                                       (m * nquad + q0 + nq) * W],
                            in_=gt2[:, 0:nq * W])

                    # ---- phase 3 for this quad pair
                    ncols = nq * QSTEP * W
                    nrows = min(nq * QSTEP, rows - QSTEP * q0)
                    gA = grpool.tile([128, 2 * QSTEP * W], BF16, tag="gA")
                    gB = grpool.tile([48, 2 * QSTEP * W], BF16, tag="gB")
                    for mm in range(8):
                        for qi in range(nq):
                            nc.sync.dma_start(
                                out=gA[16 * mm:16 * mm + 16,
                                       qi * QSTEP * W:(qi + 1) * QSTEP * W],
                                in_=gq[:, mm, q0 + qi, 0:QSTEP, :])
                    for mm in range(3):
                        for qi in range(nq):
                            nc.sync.dma_start(
                                out=gB[16 * mm:16 * mm + 16,
                                       qi * QSTEP * W:(qi + 1) * QSTEP * W],
                                in_=gq[:, 8 + mm, q0 + qi, 0:QSTEP, :])
                    obnd = opool.tile([64, 2 * QSTEP * W], F32, tag="oband")
                    nct = (ncols + 511) // 512
                    for ct in range(nct):
                        c0, c1 = ct * 512, min((ct + 1) * 512, ncols)
                        po = popool.tile([64, 512], F32, tag="po")
                        nc.tensor.matmul(po[:, 0:c1 - c0], wo[0:128, 0:64],
                                         gA[:, c0:c1],
                                         start=True, stop=False)
                        nc.tensor.matmul(po[:, 0:c1 - c0], wo[0:48, 64:128],
                                         gB[:, c0:c1],
                                         start=False, stop=True)
                        nc.vector.tensor_copy(obnd[:, c0:c1],
                                              po[:, 0:c1 - c0])
                    nc.sync.dma_start(
                        out=out_d[:, QSTEP * q0 * W:
                                  (QSTEP * q0 + nrows) * W],
                        in_=obnd[:, 0:nrows * W])

                # ------ interleaved emission: band b, then ready quad-pairs
                nbands = npr // 2
                ready = {}
                for k in range(npair):
                    quads_k = [q for q in (2 * k, 2 * k + 1) if q < nquad]
                    maxrow = QSTEP * quads_k[-1] + 14
                    b = min(nbands - 1, maxrow // 16)
                    ready.setdefault(b, []).append(k)
                for band in range(nbands):
                    emit_band(band)
                for k in range(npair):
                    emit_qg(k)

    nc.compile()
    return nc


# ----------------------------------------------------------------- interface

def _get_program(rows=ROWS):
    key = ("nc", rows)
    if key not in _cache:
        _cache[key] = build_nc(rows)
    return _cache[key]


def _patchify(xs):
    rh = xs.shape[1]
    xp = xs.reshape(DIM, rh // 8, 8, 16, 2, 8).transpose(0, 1, 3, 2, 4, 5)
    return np.ascontiguousarray(xp).reshape(DIM, rh * W).astype(
        ml_dtypes.bfloat16)


def _shard_x(x, rows=ROWS):
    rh = rows + 2 * HALO
    shards = []
    for c in range(N_CORES):
        b, hh = divmod(c, 2)
        r0 = hh * rows
        xs = np.zeros((DIM, rh, W), np.float32)
        lo, hi = r0 - HALO, r0 + rows + HALO
        slo, shi = max(lo, 0), min(hi, x.shape[2])
        xs[:, slo - lo:shi - lo] = x[b, :, slo:shi]
        shards.append(_patchify(xs))
    return shards


def _run(x, w_in, w_dw, fft_w, w_out, trace=False):
    nc = _get_program()
    wts = _prep_weights(np.asarray(w_in, np.float32),
                        np.asarray(w_dw, np.float32).reshape(C2, 3, 3),
                        np.asarray(fft_w, np.float32),
                        np.asarray(w_out, np.float32))
    shards = _shard_x(np.asarray(x, np.float32))
    in_maps = [{"x": s, **wts} for s in shards]
    res = run_bass_kernel_spmd(nc, in_maps, core_ids=list(range(N_CORES)),
                               trace=trace)
    out = np.zeros((B, DIM, H, W), np.float32)
    for c in range(N_CORES):
        b, hh = divmod(c, 2)
        out[b, :, hh * ROWS:(hh + 1) * ROWS] = (
            res.results[c]["out"].reshape(DIM, ROWS, W))
    return out, res.exec_time_ns


def kernel(x, w_in, w_dw, fft_w, w_out):
    out, _ = _run(x, w_in, w_dw, fft_w, w_out, trace=False)
    return out


# revision 34
# speedup vs baseline: 1.1419x; 1.0187x over previous
"""DFFN Trainium2 kernel for nn_DFFN_81535659147929.

Pipeline: project_in (1x1 conv, 64->340) -> per-8x8-patch rFFT2 * learned
filter -> irFFT2 -> depthwise 3x3 conv -> GELU gate -> project_out (170->64).

Key algebra: the per-patch rFFT2*w->irFFT2 step is, per channel c, a linear
map M_c on the 64 patch pixels, and all M_c are simultaneously diagonalized
by the (channel-independent) orthonormal real 2D-DFT basis C:
M_c = C^T diag(lam_c) C.  So the whole FFT stage becomes two shared-weight
matmuls around a per-(channel,freq) scale.

Sharding: data-parallel, core = (image b = core//2, row half core%2) with an
8-row patch-aligned halo; weights replicated.

Device pipeline per core:
  Phase 1, per 2-patch block (128 pixels on partitions):
    A:   psumA[pix,col] = x_block^T @ w_inT     (project_in, K=64)
    C:   psumB[frq,col] = CC @ sA               (forward transform)
    lam: sB = psumB * lam_tile                  (DVE, psum evac)
    E:   psumZ[col,pix] = sB_chunk^T @ CCrhs    (inverse transform; 4 blocks
         share a PSUM bank, one strided evac per bank -> spatial z bands,
         DMA'd to DRAM zbufs)
  Phase 2 (dwconv+gate), per (16-channel group, 8-row quad):
    partitions = (16 ch, 8 rows); the 3x3 conv becomes 3 column-shifted
    matmuls with a banded row-coupling lhsT (dense, not diagonal), psum-
    accumulated; 8 input rows yield 6 output rows (quads overlap by 2).
    Channel groups alternate x1/x2 so the GELU gate pairs align on
    partitions across the two psum tiles.  g bounced to DRAM.
  Phase 3: project_out (K=176 over two chunks) per 16-row band, DMA out.

Channel column order (NCOL=352): 22 interleaved groups of 16,
group 2m = x1 channels 16m..16m+15, group 2m+1 = x2 (= +170), last pair
padded with zero channels.
"""

import numpy as np
import ml_dtypes

import concourse.bass as bass
import concourse.mybir as mybir
from concourse import bacc, tile
from concourse.bass_utils import run_bass_kernel_spmd

BF16 = mybir.dt.bfloat16
F32 = mybir.dt.float32

DIM = 64
C2 = 340
P = 8
B, H, W = 4, 256, 256
N_CORES = 8
ROWS = H // 2
HALO = P
NCOL = 352          # 22 groups x 16, incl 12 pad channels
NGRP = 22
GCH = 176           # padded gate channels (11 groups x 16)
QSTEP = 6           # output rows per dwconv quad (8 input rows)

_cache = {}


# ----------------------------------------------------------------- host math

def _build_basis():
    rows = []
    seen = set()
    p1, p2 = np.meshgrid(np.arange(P), np.arange(P), indexing="ij")
    for u in range(P):
        for v in range(P):
            if (u, v) in seen:
                continue
            nu, nv = (-u) % P, (-v) % P
            th = 2 * np.pi * (u * p1 + v * p2) / P
            if (nu, nv) == (u, v):
                rows.append((np.cos(th) / 8.0).ravel())
            else:
                seen.add((nu, nv))
                rows.append((np.sqrt(2) / 8.0) * np.cos(th).ravel())
                rows.append((np.sqrt(2) / 8.0) * np.sin(th).ravel())
            seen.add((u, v))
    return np.array(rows, dtype=np.float64)


def _lam_for(fft_w, C):
    basis = C.reshape(64, P, P)
    F = np.fft.rfft2(basis)
    w = fft_w.reshape(C2, 1, P, P // 2 + 1).astype(np.float64)
    r = np.fft.irfft2(F[None] * w, s=(P, P))
    return np.einsum('kpq,ckpq->ck', basis, r)      # [C2, 64]


def _col_to_c2():
    """col (0..351) -> c2 channel or -1 (pad)."""
    cols = np.full(NCOL, -1, np.int64)
    for g in range(NGRP):
        m, half = divmod(g, 2)
        for j in range(16):
            ch = 16 * m + j
            if ch < 170:
                cols[g * 16 + j] = ch + 170 * half
    return cols


def _pix_maps():
    C = _build_basis()
    CCrhs = np.zeros((128, 128))
    for pc2 in range(2):
        for f in range(64):
            for p1 in range(P):
                for p2 in range(P):
                    k = p1 * 16 + pc2 * 8 + p2
                    CCrhs[pc2 * 64 + f, k] = C[f, p1 * 8 + p2]
    return CCrhs.T.copy(), CCrhs, C


def _prep_weights(w_in, w_dw, fft_w, w_out):
    CClhsT, CCrhs, C = _pix_maps()
    lam = _lam_for(fft_w, C)
    cols = _col_to_c2()
    valid = cols >= 0

    w_inT = np.zeros((64, NCOL))
    w_inT[:, valid] = w_in.T[:, cols[valid]]

    lam_t = np.zeros((128, NCOL))
    lam_sel = np.zeros((NCOL, 64))
    lam_sel[valid] = lam[cols[valid]]
    lam_t[:] = np.tile(lam_sel.T, (2, 1))[:128]

    dw = w_dw.reshape(C2, 3, 3)
    # dwconv banded lhsT per (group, dx): [128, 128] blockdiag over 16 ch of
    # T[r_in, o] = w_dw[c, r_in - o, dx], r_in - o in {0,1,2}
    dd = np.zeros((128, NGRP * 3 * 128))
    for g in range(NGRP):
        for dx in range(3):
            blk = np.zeros((128, 128))
            for j in range(16):
                c2 = cols[g * 16 + j]
                if c2 < 0:
                    continue
                for o in range(8):
                    for dy in range(3):
                        ri = o + dy
                        if ri < 8:
                            blk[j * 8 + ri, j * 8 + o] = dw[c2, dy, dx]
            dd[:, (g * 3 + dx) * 128:(g * 3 + dx + 1) * 128] = blk

    # project_out lhsT: cols 0:64 = gate-ch 0..127, cols 64:128 = 128..175
    wo = np.zeros((128, 128))
    wo[0:128, 0:64] = w_out.T[0:128]
    wo[0:42, 64:128] = w_out.T[128:170]
    bf = ml_dtypes.bfloat16
    return {
        "w_inT": w_inT.astype(bf),
        "cclhsT": CClhsT.astype(bf),
        "ccrhs": CCrhs.astype(bf),
        "lam_t": lam_t.astype(bf),
        "dd": dd.astype(bf),
        "wo": wo.astype(bf),
    }


# ---------------------------------------------------------------- bass build

def build_nc(rows=ROWS, dbg=False):
    rh = rows + 2 * HALO
    npr = rh // P
    nband = rows // 16
    nquad = (rows + QSTEP - 1) // QSTEP          # 22 for rows=128
    assert rows % 16 == 0

    nc = bacc.Bacc("TRN2", target_bir_lowering=False, debug=False,
                   num_devices=N_CORES)
    x_d = nc.dram_tensor("x", [DIM, rh * W], BF16, kind="ExternalInput")
    winT_d = nc.dram_tensor("w_inT", [64, NCOL], BF16, kind="ExternalInput")
    cclhsT_d = nc.dram_tensor("cclhsT", [128, 128], BF16, kind="ExternalInput")
    ccrhs_d = nc.dram_tensor("ccrhs", [128, 128], BF16, kind="ExternalInput")
    lam_d = nc.dram_tensor("lam_t", [128, NCOL], BF16, kind="ExternalInput")
    dd_d = nc.dram_tensor("dd", [128, NGRP * 3 * 128], BF16,
                          kind="ExternalInput")
    wo_d = nc.dram_tensor("wo", [128, 128], BF16, kind="ExternalInput")
    out_d = nc.dram_tensor("out", [DIM, rows * W], F32, kind="ExternalOutput")

    # z bounce buffers, one per 128-col chunk of the NCOL channel columns
    zkind = "ExternalOutput" if dbg else "Internal"
    zbufs = [nc.dram_tensor(f"zbuf{i}", [128, rh * W], BF16, kind=zkind)
             for i in range(3)]          # chunks: 128, 128, 96 rows used
    gbuf_d = nc.dram_tensor("gbuf", [128, 11 * nquad * W], BF16, kind=zkind)

    G = mybir.ActivationFunctionType.Gelu
    chunks = [(0, 128), (128, 128), (256, 96)]

    with tile.TileContext(nc) as tc:
        with tc.tile_pool(name="consts", bufs=1) as cpool:
            w_inT = cpool.tile([64, NCOL], BF16)
            nc.sync.dma_start(out=w_inT[:], in_=winT_d[:])
            cclhsT = cpool.tile([128, 128], BF16)
            nc.sync.dma_start(out=cclhsT[:], in_=cclhsT_d[:])
            ccrhs = cpool.tile([128, 128], BF16)
            nc.sync.dma_start(out=ccrhs[:], in_=ccrhs_d[:])
            lam_t = cpool.tile([128, NCOL], BF16)
            nc.sync.dma_start(out=lam_t[:], in_=lam_d[:])
            dd = cpool.tile([128, NGRP * 3 * 128], BF16)
            nc.sync.dma_start(out=dd[:], in_=dd_d[:])
            wo = cpool.tile([128, 128], BF16)
            nc.sync.dma_start(out=wo[:], in_=wo_d[:])

            # ---------------- all pools coexist: PSUM adds up to 8 banks
            dxs = [(1, 0, 0, 256), (0, 0, 1, 255), (2, 1, 0, 255)]
            gq = gbuf_d[:].rearrange("(j o) (m q w) -> j m q o w",
                                     j=16, o=8, m=11, q=nquad, w=W)
            npair = (nquad + 1) // 2
            with (
                tc.tile_pool(name="p1x", bufs=1) as xpool,
                tc.tile_pool(name="p1s", bufs=4) as spool,
                tc.tile_pool(name="p1sb", bufs=8) as sbpool,
                tc.tile_pool(name="p1z", bufs=2) as zpool,
                tc.tile_pool(name="p1ps", bufs=2, space="PSUM") as pspool,
                tc.tile_pool(name="p1pz", bufs=2, space="PSUM") as pzpool,
                tc.tile_pool(name="p2z", bufs=4) as zqpool,
                tc.tile_pool(name="p2g", bufs=4) as gpool,
                tc.tile_pool(name="p2ps", bufs=1, space="PSUM") as qpool,
                tc.tile_pool(name="p3g", bufs=2) as grpool,
                tc.tile_pool(name="p3o", bufs=1) as opool,
                tc.tile_pool(name="p3ps", bufs=2, space="PSUM") as popool,
            ):
                # x arrives host-patchified: [64, (pr, pcp, p1, pc2, p2)]
                x_sb = xpool.tile([64, rh * W], BF16)
                bsz = 32 * 128      # one 2-patch-row band of columns
                for bb in range(npr // 2):
                    nc.sync.dma_start(out=x_sb[:, bb * bsz:(bb + 1) * bsz],
                                      in_=x_d[:, bb * bsz:(bb + 1) * bsz])

                # HAM warmup: dense dummy matmuls while the first DMAs land
                wps = pspool.tile([128, NCOL], F32, tag="pAB", name="warm")
                for _ in range(40):
                    nc.tensor.matmul(wps[:, 0:128], cclhsT[:], ccrhs[:],
                                     start=True, stop=True)

                def emit_band(band):
                    prs = [2 * band, 2 * band + 1]
                    zbs = [zpool.tile([128, 16 * W], BF16, tag=f"zb{i}",
                                      name=f"zb{i}")
                           for i in range(3)]
                    for bi, pr in enumerate(prs):
                        for pq in range(4):          # 4 blocks of 4 pcp
                            sBs = []
                            for pj in range(4):
                                pcp = pq * 4 + pj
                                psA = pspool.tile([128, NCOL], F32, tag="pAB")
                                xblk = x_sb[:, (pr * 16 + pcp) * 128:
                                            (pr * 16 + pcp + 1) * 128]
                                nc.tensor.matmul(psA[:], xblk, w_inT[:],
                                                 start=True, stop=True)
                                sA = spool.tile([128, NCOL], BF16, tag="sA")
                                if pj % 2 == 0:
                                    nc.scalar.copy(sA[:], psA[:])
                                else:
                                    nc.vector.tensor_copy(sA[:], psA[:])
                                psB = pspool.tile([128, NCOL], F32, tag="pAB",
                                                  name="psB")
                                nc.tensor.matmul(psB[:], cclhsT[:], sA[:],
                                                 start=True, stop=True)
                                sB = sbpool.tile([128, NCOL], BF16, tag="sB")
                                nc.vector.tensor_mul(sB[:], psB[:], lam_t[:])
                                sBs.append(sB)
                            for ci, (c0, m) in enumerate(chunks):
                                psZ = pzpool.tile([128, 512], F32,
                                                  tag="psZ")
                                for pj in range(4):
                                    nc.tensor.matmul(
                                        psZ[0:m, pj * 128:(pj + 1) * 128],
                                        sBs[pj][:, c0:c0 + m], ccrhs[:],
                                        start=(pj == 0), stop=(pj == 3))
                                # evac: psum cols (pj, p1, pc2p2) read as
                                # (p1, pj, pc2p2) -> zband rows/cols
                                src = psZ[:].rearrange(
                                    "c (pj p1 q) -> c p1 pj q",
                                    pj=4, p1=8, q=16)[0:m]
                                dstr = zbs[ci][:].rearrange(
                                    "c (r pcp q) -> c r pcp q",
                                    r=16, pcp=16, q=16)
                                dst = dstr[0:m, bi * 8:bi * 8 + 8,
                                           pq * 4:pq * 4 + 4, :]
                                if (pq + ci) % 2 == 0:
                                    nc.scalar.copy(dst, src)
                                else:
                                    nc.vector.tensor_copy(dst, src)
                    r0 = 2 * band * 8 * W
                    for i in range(3):
                        nc.sync.dma_start(out=zbufs[i][:, r0:r0 + 16 * W],
                                          in_=zbs[i][:])

                # ------ phases 2+3 interleaved over quad-pair groups
                def emit_qg(qg):
                    q0 = 2 * qg
                    quads = [q for q in (q0, q0 + 1) if q < nquad]
                    nq = len(quads)
                    for m in range(11):              # gate pair m
                        pss = []
                        for half in range(2):
                            g = 2 * m + half
                            ci, roff = g // 8, (g % 8) * 16
                            zt2 = zqpool.tile([128, 2 * W], BF16,
                                              tag=f"zq{half}",
                                              name=f"zq{half}")
                            zsrc = zbufs[ci][:].rearrange(
                                "c (r w) -> c r w", r=rh, w=W)
                            eng = nc.gpsimd
                            for qi, q in enumerate(quads):
                                eng.dma_start(
                                    out=zt2[:, qi * W:(qi + 1) * W],
                                    in_=zsrc[roff:roff + 16,
                                             QSTEP * q + 7:QSTEP * q + 15, :])
                            ps = qpool.tile([128, 2 * W], F32,
                                            tag=f"ps{half}", name=f"ps{half}")
                            zr = zt2[:].rearrange("c (q w) -> c q w", q=2, w=W)
                            pr = ps[:].rearrange("c (q w) -> c q w", q=2, w=W)
                            for k, (dx, wi0, wo0, wn) in enumerate(dxs):
                                lhs = dd[:, (g * 3 + dx) * 128:
                                         (g * 3 + dx + 1) * 128]
                                nc.tensor.matmul(
                                    pr[:, 0:nq, wo0:wo0 + wn], lhs,
                                    zr[:, 0:nq, wi0:wi0 + wn],
                                    start=(k == 0), stop=(k == 2))
                            pss.append(ps)
                        ge = gpool.tile([128, 2 * W], BF16, tag="ge")
                        nc.scalar.activation(ge[:, 0:nq * W],
                                             pss[0][:, 0:nq * W], G)
                        gt2 = gpool.tile([128, 2 * W], BF16, tag="gt2")
                        nc.vector.tensor_mul(gt2[:, 0:nq * W],
                                             ge[:, 0:nq * W],
                                             pss[1][:, 0:nq * W])
                        nc.sync.dma_start(
                            out=gbuf_d[:, (m * nquad + q0) * W:
                                       (m * nquad + q0 + nq) * W],
                            in_=gt2[:, 0:nq * W])

                    # ---- phase 3 for this quad pair
                    ncols = nq * QSTEP * W
                    nrows = min(nq * QSTEP, rows - QSTEP * q0)
                    gA = grpool.tile([128, 2 * QSTEP * W], BF16, tag="gA")
                    gB = grpool.tile([48, 2 * QSTEP * W], BF16, tag="gB")
                    for mm in range(8):
                        for qi in range(nq):
                            nc.sync.dma_start(
                                out=gA[16 * mm:16 * mm + 16,
                                       qi * QSTEP * W:(qi + 1) * QSTEP * W],
                                in_=gq[:, mm, q0 + qi, 0:QSTEP, :])
                    for mm in range(3):
                        for qi in range(nq):
                            nc.sync.dma_start(
                                out=gB[16 * mm:16 * mm + 16,
                                       qi * QSTEP * W:(qi + 1) * QSTEP * W],
                                in_=gq[:, 8 + mm, q0 + qi, 0:QSTEP, :])
                    obnd = opool.tile([64, 2 * QSTEP * W], F32, tag="oband")
                    nct = (ncols + 511) // 512
                    for ct in range(nct):
                        c0, c1 = ct * 512, min((ct + 1) * 512, ncols)
                        po = popool.tile([64, 512], F32, tag="po")
                        nc.tensor.matmul(po[:, 0:c1 - c0], wo[0:128, 0:64],
                                         gA[:, c0:c1],
                                         start=True, stop=False)
                        nc.tensor.matmul(po[:, 0:c1 - c0], wo[0:48, 64:128],
                                         gB[:, c0:c1],
                                         start=False, stop=True)
                        nc.vector.tensor_copy(obnd[:, c0:c1],
                                              po[:, 0:c1 - c0])
                    nc.sync.dma_start(
                        out=out_d[:, QSTEP * q0 * W:
                                  (QSTEP * q0 + nrows) * W],
                        in_=obnd[:, 0:nrows * W])

                # ------ interleaved emission: band b, then ready quad-pairs
                nbands = npr // 2
                ready = {}
                for k in range(npair):
                    quads_k = [q for q in (2 * k, 2 * k + 1) if q < nquad]
                    maxrow = QSTEP * quads_k[-1] + 14
                    b = min(nbands - 1, maxrow // 16)
                    ready.setdefault(b, []).append(k)
                for band in range(nbands):
                    emit_band(band)
                for k in range(npair):
                    emit_qg(k)

    nc.compile()
    return nc


# ----------------------------------------------------------------- interface

def _get_program(rows=ROWS):
    key = ("nc", rows)
    if key not in _cache:
        _cache[key] = build_nc(rows)
    return _cache[key]


def _patchify(xs):
    rh = xs.shape[1]
    xp = xs.reshape(DIM, rh // 8, 8, 16, 2, 8).transpose(0, 1, 3, 2, 4, 5)
    return np.ascontiguousarray(xp).reshape(DIM, rh * W).astype(
        ml_dtypes.bfloat16)


def _shard_x(x, rows=ROWS):
    rh = rows + 2 * HALO
    shards = []
    for c in range(N_CORES):
        b, hh = divmod(c, 2)
        r0 = hh * rows
        xs = np.zeros((DIM, rh, W), np.float32)
        lo, hi = r0 - HALO, r0 + rows + HALO
        slo, shi = max(lo, 0), min(hi, x.shape[2])
        xs[:, slo - lo:shi - lo] = x[b, :, slo:shi]
        shards.append(_patchify(xs))
    return shards


def _run(x, w_in, w_dw, fft_w, w_out, trace=False):
    nc = _get_program()
    wts = _prep_weights(np.asarray(w_in, np.float32),
                        np.asarray(w_dw, np.float32).reshape(C2, 3, 3),
                        np.asarray(fft_w, np.float32),
                        np.asarray(w_out, np.float32))
    shards = _shard_x(np.asarray(x, np.float32))
    in_maps = [{"x": s, **wts} for s in shards]
    res = run_bass_kernel_spmd(nc, in_maps, core_ids=list(range(N_CORES)),
                               trace=trace)
    out = np.zeros((B, DIM, H, W), np.float32)
    for c in range(N_CORES):
        b, hh = divmod(c, 2)
        out[b, :, hh * ROWS:(hh + 1) * ROWS] = (
            res.results[c]["out"].reshape(DIM, ROWS, W))
    return out, res.exec_time_ns


def kernel(x, w_in, w_dw, fft_w, w_out):
    out, _ = _run(x, w_in, w_dw, fft_w, w_out, trace=False)
    return out
